# revision 1
# baseline (speedup 1.0000x reference)
"""CapsuleLayer (dynamic routing) Trainium2 kernel.

Math: reference routing updates B_logits += exp(-d2) where
d2 = |prior - out|^2 per (b, c, r). For these magnitudes d2 is ~chi^2
distributed around 128, so exp(-d2) underflows reference f32 for all but a
sparse set of triples (d2 < ~19 is the f32-visible cutoff). Device computes
  s_sum[b,c,o] = sum_r priors[b,c,r,o]        (exact f32 matmul)
  q[b,c,r]     = |priors[b,c,r,:]|_1          (bf16 block-diag matmul + abs
                                               reduce; threshold only —
                                               Cauchy-Schwarz bounds L2)
R-sharded over 8 cores (zero input replication). Host gathers, finds the
sparse set {q < THETA}, recomputes those priors exactly in f64, and runs the
exact 3-iteration routing with sparse softmax corrections.
"""

import sys
import functools

sys.path.insert(0, "/opt/trn_rl_repo")

import numpy as np
import ml_dtypes

B, C, R, I, O = 128, 10, 4608, 8, 16
NCORES = 8
RL = R // NCORES            # 576 route nodes per core
RCHUNK = RL // 16           # 36 chunks of 16 r (=128 contraction rows)
ROUTE_ITERATIONS = 3
SIGMA = 1.0
THETA = 20.0                # |p|_1 threshold: d2<20 => |p|_1 < 18.6 (C-S)

LAST_RESULTS = None         # BassKernelResults of the most recent run (for test)


def _build_nc(reps=1, parts="full"):
    import concourse.bass as bass
    import concourse.mybir as mybir
    from concourse.tile import TileContext
    from concourse.masks import make_identity

    f32 = mybir.dt.float32
    bf16 = mybir.dt.bfloat16
    CO = C * O              # 160
    NB = 256                # block-diag matmul free size = 16 r * 16 o

    nc = bass.Bass(trn_type="TRN2")
    xs = nc.dram_tensor("xs", [B, RL * I], f32, kind="ExternalInput")
    ws = nc.dram_tensor("ws", [RCHUNK, 128, CO], f32, kind="ExternalInput")
    mk = nc.dram_tensor("mk", [128, C * NB], f32, kind="ExternalInput")
    s_out = nc.dram_tensor("s_out", [B, CO], f32, kind="ExternalOutput")
    q_out = nc.dram_tensor("q_out", [RCHUNK, B, C * 16], f32, kind="ExternalOutput")

    GRP = 6                 # rc chunks per W preload DMA group

    with TileContext(nc) as tc:
        with (
            tc.tile_pool(name="const", bufs=1) as constp,
            tc.tile_pool(name="wblk", bufs=2) as wblkp,
            tc.tile_pool(name="sq", bufs=RCHUNK) as sqp,
            tc.tile_pool(name="qsb", bufs=RCHUNK // 2) as qsbp,
            tc.tile_pool(name="ps_s", bufs=1, space="PSUM") as ps_s,
            tc.tile_pool(name="ps_t", bufs=2, space="PSUM") as ps_t,
            tc.tile_pool(name="ps_p", bufs=2, space="PSUM") as ps_p,
            tc.tile_pool(name="ps_q", bufs=1, space="PSUM") as ps_q,
        ):
            ident = constp.tile([128, 128], f32)
            make_identity(nc, ident[:])
            mask = constp.tile([128, C * NB], f32)
            nc.sync.dma_start(mask[:], mk[:])
            xs_sb = constp.tile([B, RL * I], f32)
            nc.sync.dma_start(xs_sb[:], xs[:])

            # Preload all W in GRP-chunk group DMAs (independent tiles).
            ws_g = []
            for g in range(RCHUNK // GRP):
                wsg = constp.tile([128, GRP * CO], f32, tag=f"wsg{g}")
                nc.sync.dma_start(
                    wsg[:].rearrange("p (rc co) -> p rc co", rc=GRP),
                    ws[g * GRP:(g + 1) * GRP].rearrange("rc p co -> p rc co"),
                )
                ws_g.append(wsg)

            # PE wait-absorbers: any instruction carries at most one
            # sync-wait, so observe the identity (Pool) and xs (DMA)
            # semaphores on dummy bf16 ldweights (no PSUM write; every real
            # matmul re-embeds its own weight load).
            nc.tensor.ldweights(ident[:, 0:64].bitcast(bf16))
            nc.tensor.ldweights(xs_sb[:, 0:64].bitcast(bf16))

            s_psum = ps_s.tile([128, CO], f32)

            for rep in range(reps):
                # ---- Phase 1: transpose all x chunks, one bulk bf16 cast ----
                xt = constp.tile([128, RL * I], f32, tag="xt")
                for rcp in range(RCHUNK // 2):
                    tp = ps_t.tile([128, 256], f32, tag="tp")
                    for k in range(2):
                        rc = rcp * 2 + k
                        nc.tensor.transpose(
                            tp[:, k * 128:(k + 1) * 128],
                            xs_sb[:, rc * 128:(rc + 1) * 128], ident[:])
                    nc.vector.tensor_copy(
                        xt[:, rcp * 256:(rcp + 1) * 256], tp[:])
                xt16 = constp.tile([128, RL * I], bf16, tag="xt16")
                nc.scalar.copy(xt16[:], xt[:])
                # Let the PE observe the bulk cast once (ACT) and DVE (xt).
                nc.tensor.ldweights(xt16[:, 0:128])

                # ---- Phase 2: s-matmuls + block-diag q pipeline ----
                for rc2 in range(RCHUNK // 2):
                    qsb = qsbp.tile([B, 2 * C * 16], f32)
                    # pair-merged block-diagonal construction (one DVE op)
                    rc0 = rc2 * 2
                    g, gi = divmod(rc0, GRP)
                    wsg = ws_g[g]
                    if gi == 0:
                        # Absorb this W group's DMA semaphore.
                        nc.tensor.ldweights(wsg[:, 0:64].bitcast(bf16))
                    wblk = wblkp.tile([128, 2 * C * NB], bf16)
                    w_b = (
                        wsg[:, gi * CO:(gi + 2) * CO]
                        .rearrange("p (rc c o) -> p rc c o", rc=2, c=C)
                        .unsqueeze(3)
                        .broadcast_to((128, 2, C, 16, O))
                    )
                    m_b = (
                        mask[:].rearrange("p (c r o) -> p c r o", c=C, r=16)
                        .unsqueeze(1)
                        .broadcast_to((128, 2, C, 16, O))
                    )
                    nc.vector.tensor_tensor(
                        wblk[:].rearrange(
                            "p (rc c r o) -> p rc c r o", rc=2, c=C, r=16),
                        w_b, m_b, mybir.AluOpType.mult,
                    )
                    for k in range(2):
                        rc = rc0 + k
                        xt_sl = xt[:, rc * 128:(rc + 1) * 128]
                        nc.tensor.matmul(
                            s_psum[:], xt_sl, wsg[:, (gi + k) * CO:(gi + k + 1) * CO],
                            start=(rc == 0), stop=(rc == RCHUNK - 1),
                            skip_group_check=True,
                        )
                        xt16_sl = xt16[:, rc * 128:(rc + 1) * 128]
                        wb0 = k * C * NB
                        qb0 = k * C * 16
                        for half in range(2):
                            # 4 capsules per 2-bank PSUM tile, one L1 reduce
                            pp = ps_p.tile([128, 4 * NB], f32, tag="pp")
                            for j in range(2):
                                c0 = half * 4 + j * 2
                                nc.tensor.matmul(
                                    pp[:, j * 2 * NB:(j + 1) * 2 * NB],
                                    xt16_sl,
                                    wblk[:, wb0 + c0 * NB:wb0 + (c0 + 2) * NB],
                                    start=True, stop=True,
                                    skip_group_check=True,
                                )
                            if parts == "pmm":
                                continue
                            nc.vector.tensor_reduce(
                                qsb[:, qb0 + half * 64:qb0 + half * 64 + 64],
                                pp[:].rearrange(
                                    "p (cc r o) -> p cc r o", cc=4, o=O),
                                mybir.AxisListType.X,
                                mybir.AluOpType.add,
                                apply_absolute_value=True,
                            )
                        # leftover capsules 8,9
                        pps = ps_q.tile([128, 2 * NB], f32, tag="pps")
                        nc.tensor.matmul(
                            pps[:], xt16_sl,
                            wblk[:, wb0 + 8 * NB:wb0 + 10 * NB],
                            start=True, stop=True, skip_group_check=True,
                        )
                        if parts != "pmm":
                            nc.vector.tensor_reduce(
                                qsb[:, qb0 + 128:qb0 + 160],
                                pps[:].rearrange(
                                    "p (cc r o) -> p cc r o", cc=2, o=O),
                                mybir.AxisListType.X,
                                mybir.AluOpType.add,
                                apply_absolute_value=True,
                            )
                    if parts != "pmm":
                        nc.sync.dma_start(
                            q_out[rc0:rc0 + 2].rearrange("rc b f -> b rc f"),
                            qsb[:].rearrange("b (rc f) -> b rc f", rc=2),
                        )

            s_sb = constp.tile([B, CO], f32)
            nc.vector.tensor_copy(s_sb[:], s_psum[:])
            nc.sync.dma_start(s_out[:], s_sb[:])

    _split_multi_waits(nc)
    return nc


def _split_multi_waits(nc):
    """Walrus codegen accepts at most one sync-wait per instruction; hoist
    extra waits onto preceding same-engine NoOps (semantically identical —
    the engine stalls at the NoOp instead)."""
    import bass_rust

    for func in nc.m.functions:
        for blk in func.blocks:
            insts = blk.instructions
            new_list = []
            n_split = 0
            for inst in insts:
                si = getattr(inst, "sync_info", None)
                waits = list(si.on_wait) if si is not None else []
                if len(waits) > 1:
                    for k, w in enumerate(waits[:-1]):
                        no = bass_rust.InstNoOp(name=f"{inst.name}-ws{k}")
                        no.engine = inst.engine
                        no.sync_info = bass_rust.SyncInfo(
                            on_wait=[w], on_update=[]
                        )
                        new_list.append(no)
                        n_split += 1
                    inst.sync_info = bass_rust.SyncInfo(
                        on_wait=[waits[-1]], on_update=list(si.on_update)
                    )
                new_list.append(inst)
            if n_split:
                blk.instructions = new_list


@functools.lru_cache(maxsize=8)
def _get_nc(reps=1, parts="full"):
    return _build_nc(reps, parts)


@functools.lru_cache(maxsize=1)
def _get_mask():
    m = np.zeros((128, C, 16, O), dtype=np.float32)
    for p in range(128):
        m[p, :, p // 8, :] = 1
    return np.ascontiguousarray(m.reshape(128, C * 16 * O))


def _squash64(s):
    sq = (s * s).sum(-1, keepdims=True)
    return (sq / (1.0 + sq)) * s / np.sqrt(sq)


def kernel(x, route_weights, capsule_bias):
    global LAST_RESULTS
    from concourse.bass_utils import run_bass_kernel_spmd

    x = np.ascontiguousarray(np.asarray(x, dtype=np.float32))
    W = np.ascontiguousarray(np.asarray(route_weights, dtype=np.float32))
    bias = np.asarray(capsule_bias, dtype=np.float64).reshape(C, O)

    mask = _get_mask()
    in_maps = []
    for k in range(NCORES):
        rs, re = k * RL, (k + 1) * RL
        xs_k = x[:, rs:re, :].reshape(B, RL * I)
        # [C, RL, I, O] -> [RCHUNK, (16r 8i), (c o)]
        ws_k = np.ascontiguousarray(
            W[:, rs:re]
            .reshape(C, RCHUNK, 16, I, O)
            .transpose(1, 2, 3, 0, 4)
            .reshape(RCHUNK, 128, C * O)
        )
        in_maps.append({"xs": np.ascontiguousarray(xs_k), "ws": ws_k, "mk": mask})

    res = run_bass_kernel_spmd(_get_nc(), in_maps, core_ids=list(range(NCORES)))
    LAST_RESULTS = res
    outs = res.results

    s_sum = np.zeros((B, C, O), dtype=np.float64)
    q = np.empty((B, C, R), dtype=np.float32)
    for k in range(NCORES):
        s_sum += np.asarray(outs[k]["s_out"], dtype=np.float64).reshape(B, C, O)
        # q_out: [RCHUNK, B, (c rl)] -> [B, C, RL]
        qk = np.asarray(outs[k]["q_out"]).reshape(RCHUNK, B, C, 16)
        q[:, :, k * RL:(k + 1) * RL] = (
            qk.transpose(1, 2, 0, 3).reshape(B, C, RL)
        )

    # ---- host sparse routing correction (exact, f64) ----
    bs, cs, rs_ = np.nonzero(q < THETA)
    pS = np.einsum(
        "si,sio->so",
        x[bs, rs_].astype(np.float64),
        W[cs, rs_].astype(np.float64),
    )
    qS = (pS * pS).sum(-1)

    L = np.zeros(len(bs), dtype=np.float64)
    out = None
    for it in range(ROUTE_ITERATIONS):
        u = np.expm1(L)
        usum = np.zeros((B, C))
        np.add.at(usum, (bs, cs), u)
        corr = np.zeros((B, C, O))
        np.add.at(corr, (bs, cs), u[:, None] * pS)
        s = (s_sum + corr) / (R + usum)[..., None]
        out = _squash64(s) + bias[None]
        if it < ROUTE_ITERATIONS - 1:
            outS = out[bs, cs]
            d2 = qS - 2.0 * (pS * outS).sum(-1) + (outS * outS).sum(-1)
            L = L + np.exp(-d2 / (SIGMA * SIGMA))

    return out.astype(np.float32)



# revision 3
# speedup vs baseline: 1.7819x; 1.7819x over previous
"""CapsuleLayer (dynamic routing) Trainium2 kernel, v2.

Math: the routing update B_logits += exp(-d2), d2 = |prior - out|^2, is
dominated by underflow: priors have |p|^2 ~ chi^2_16 * 8 (mean ~128), so
exp(-d2) is negligible except where |p| is small. Device computes
  s_sum[b,c,o] = sum_r priors[b,c,r,o]     (bf16 matmul chain, f32 PSUM)
  q2[b,c,r]   ~= |p[b,c,r,:]|_2^2          (screen only, via Cholesky:
                                            G=W W^T = L L^T per (c,r),
                                            z = x L, q2 = |z|^2)
R-sharded over 8 cores. Host gathers, finds the sparse set {q2 < THETA2},
recomputes those priors exactly in f64, and runs the exact 3-iteration
routing with sparse softmax corrections.

Device pipeline per 16-route chunk (rc): 3 block-diag matmuls (bf16 x
fp8) + 1 s-matmul (bf16) on PE, then one of two engine paths (DVE cannot
read two PSUM operands, so squares run on ACT only):
  z-chunks (ACT): pp = z = xL; ACT squares psum->sbuf bf16; DVE pair-adds
    the m-planes (2x bf16 mode).
  y-chunks (DVE): pp = y = Gx; DVE mult pp*x (one PSUM input) -> x_i*y_i
    bf16; DVE pair-adds the i-planes.
Both ship 4 partial sums per (b,c,r) as fp8; host finishes the 4-way sum,
so fp8's 6% granularity only widens the screen threshold.
"""

import sys
import functools

sys.path.insert(0, "/opt/trn_rl_repo")

import numpy as np
import ml_dtypes

B, C, R, I, O = 128, 10, 4608, 8, 16
M = 8                       # Cholesky factor width (G is 8x8, full rank)
NCORES = 8
RL = R // NCORES            # 576 route nodes per core
RCHUNK = RL // 16           # 36 chunks of 16 r (=128 contraction rows)
ZW = M * C * 16             # z columns per rc chunk: (m, c, r) = 1280
VW = (M // 2) * C * 16      # v columns per rc chunk after pair-add = 640
CO = C * O                  # 160
ROUTE_ITERATIONS = 3
SIGMA = 1.0
THETA2 = 42.0               # |p|^2 screen threshold (true cut needs ~29.2)
EPS_JITTER = 1e-3           # G + eps*I before Cholesky
# Engine split: 5 y-form chunks on DVE, spread mid-stream (after the gb[0]
# prefetch lands, before the tail whose latency gates the last v slab);
# the rest are z-form on ACT. Balances ACT ~31us vs DVE ~30us.
Y_CHUNKS = (8, 14, 20, 26, 31)
ACT_CHUNK = [rc not in Y_CHUNKS for rc in range(RCHUNK)]
Y_INDEX = {rc: i for i, rc in enumerate(Y_CHUNKS)}
SCALE_Y = 8.0               # y-form factor pre-scale (fp8 overflow headroom)

# z-form uses the best rank-MZ PSD approximation of G (eigh truncation):
# |x L6|^2 <= x G x^T keeps the rejection certificate exact; the screen just
# keeps slightly more triples (residual eigenvalues ~4-5 on a ~128 scale).
MZ = 6
ZW_RC = [ZW if rc in Y_CHUNKS else MZ * C * 16 for rc in range(RCHUNK)]
VW_RC = [(ZW_RC[rc] // 2) for rc in range(RCHUNK)]
GOFF = np.concatenate(([0], np.cumsum(ZW_RC))).astype(int)
VOFF = np.concatenate(([0], np.cumsum(VW_RC))).astype(int)
GB_COLS = int(GOFF[-1])     # 36160
V_COLS = int(VOFF[-1])      # 18080
# gb DMA groups: small first group so compute starts early
GB_GROUPS = ((0, 2), (2, 6), (6, 12), (12, 18), (18, 24), (24, 30), (30, 36))
V_SLABS = ((0, 8), (8, 16), (16, 24), (24, 32), (32, 36))

LAST_RESULTS = None         # BassKernelResults of the most recent run (for test)


def _build_nc(reps=1):
    import concourse.bass as bass
    import concourse.mybir as mybir
    from concourse.tile import TileContext

    f32 = mybir.dt.float32
    bf16 = mybir.dt.bfloat16
    fp8 = mybir.dt.float8e4

    GRP = 6                 # rc chunks per Gblk DMA group
    NGRP = RCHUNK // GRP

    nc = bass.Bass(trn_type="TRN2")
    NY = len(Y_CHUNKS)
    xt = nc.dram_tensor("xt", [128, RCHUNK * B], bf16, kind="ExternalInput")
    xs = nc.dram_tensor("xs", [B, NY * 128], bf16, kind="ExternalInput")
    ws = nc.dram_tensor("ws", [128, RCHUNK * CO], bf16, kind="ExternalInput")
    gb = nc.dram_tensor("gb", [128, GB_COLS], fp8, kind="ExternalInput")
    s_out = nc.dram_tensor("s_out", [B, CO], f32, kind="ExternalOutput")
    v_out = nc.dram_tensor("v_out", [B, V_COLS], fp8, kind="ExternalOutput")

    with TileContext(nc) as tc:
        with (
            tc.tile_pool(name="const", bufs=1) as constp,
            tc.tile_pool(name="gblk", bufs=3) as gblkp,
            tc.tile_pool(name="sq", bufs=3) as sqp,
            tc.tile_pool(name="ps_s", bufs=1, space="PSUM") as ps_s,
            tc.tile_pool(name="ps_p", bufs=2, space="PSUM") as ps_p,
        ):
            # DMA issue order tracks the critical path: the z/y screen
            # pipeline needs xt + gb groups first; ws (s-matmuls) is pure
            # fill-in PE work and goes last. Everything fits in SBUF, so gb
            # lives in one resident tile filled by a few grouped DMAs.
            xt_sb = constp.tile([128, RCHUNK * B], bf16)
            nc.sync.dma_start(xt_sb[:], xt[:])
            vsb = constp.tile([B, V_COLS], fp8)

            gb_tiles = {}

            def _gb_group_dma(gidx):
                lo, hi = GB_GROUPS[gidx]
                gt = constp.tile(
                    [128, int(GOFF[hi] - GOFF[lo])], fp8, tag=f"gbg{gidx}")
                nc.sync.dma_start(gt[:], gb[:, GOFF[lo]:GOFF[hi]])
                for rc in range(lo, hi):
                    gb_tiles[rc] = (gt, int(GOFF[rc] - GOFF[lo]))

            _gb_group_dma(0)
            xs_sb = constp.tile([B, NY * 128], bf16)
            nc.sync.dma_start(xs_sb[:], xs[:])
            for gidx in range(1, len(GB_GROUPS)):
                _gb_group_dma(gidx)

            ws_sb = constp.tile([128, RCHUNK * CO], bf16)
            nc.sync.dma_start(ws_sb[:], ws[:])

            s_psum = ps_s.tile([128, CO], f32)

            for rep in range(reps):
                for rc in range(RCHUNK):
                    gt, goff = gb_tiles[rc]
                    zw = ZW_RC[rc]
                    nm = zw // (C * 16)         # m-planes this chunk (6 or 8)
                    xt_sl = xt_sb[:, rc * B:(rc + 1) * B]
                    # s-matmul: accumulate sum_r priors over all chunks
                    nc.tensor.matmul(
                        s_psum[:], xt_sl, ws_sb[:, rc * CO:(rc + 1) * CO],
                        start=(rep == 0 and rc == 0),
                        stop=(rep == reps - 1 and rc == RCHUNK - 1),
                        skip_group_check=True,
                    )
                    # screen matmuls: pp[b, (m,c,r)] = x . factor (block-diag)
                    pp = ps_p.tile([128, ZW], f32)
                    for c0 in range(0, zw, 512):
                        c1 = min(c0 + 512, zw)
                        nc.tensor.matmul(
                            pp[:, c0:c1], xt_sl,
                            gt[:, goff + c0:goff + c1],
                            start=True, stop=True, skip_group_check=True,
                        )
                    # per-element screen terms -> sq (bf16, SBUF)
                    sq = sqp.tile([128, ZW], bf16)
                    with nc.allow_low_precision("screen-only q2 path"):
                        if ACT_CHUNK[rc]:
                            # z-form: sq = z^2 on ACT
                            nc.scalar.square(sq[:, 0:zw], pp[:, 0:zw])
                        else:
                            # y-form: sq = y * x on DVE (one PSUM input)
                            yi = Y_INDEX[rc]
                            xb = (
                                xs_sb[:, yi * 128:(yi + 1) * 128]
                                .rearrange("p (r i) -> p i r", i=I)
                                .unsqueeze(2)
                                .broadcast_to((B, I, C, 16))
                            )
                            nc.vector.tensor_tensor(
                                sq[:, 0:zw].rearrange(
                                    "p (i c r) -> p i c r", i=I, c=C),
                                pp[:, 0:zw].rearrange(
                                    "p (i c r) -> p i c r", i=I, c=C),
                                xb, mybir.AluOpType.mult)
                        # pair-add over planes: v[m'] = sq[2m'] + sq[2m'+1]
                        sq4 = sq[:, 0:zw].rearrange(
                            "p (m2 mp cr) -> p m2 mp cr", m2=nm // 2, mp=2)
                        nc.vector.tensor_tensor(
                            vsb[:, VOFF[rc]:VOFF[rc + 1]].rearrange(
                                "p (m2 cr) -> p m2 cr", m2=nm // 2),
                            sq4[:, :, 0], sq4[:, :, 1], mybir.AluOpType.add)

                # v out in slabs (inside rep loop: the in-NEFF repetition
                # slope then covers the full steady-state cost); the final
                # slab is small — it sits on the critical tail.
                for lo, hi in V_SLABS:
                    nc.sync.dma_start(
                        v_out[:, VOFF[lo]:VOFF[hi]],
                        vsb[:, VOFF[lo]:VOFF[hi]],
                    )
            s_sb = constp.tile([B, CO], f32)
            nc.vector.tensor_copy(s_sb[:], s_psum[:])
            nc.sync.dma_start(s_out[:], s_sb[:])

    _dedupe_ldweights(nc)
    _split_multi_waits(nc)
    return nc


def _dedupe_ldweights(nc):
    """Drop an InstLdweights that reloads exactly the weights loaded by the
    immediately preceding InstLdweights (PE weights are sticky between
    loads). Only sync-free duplicates are dropped."""
    import bass_rust

    def sig(inst):
        a = inst.ins[0]
        return (str(getattr(a, "memorylocation", None)), a.offset, str(a.ap))

    for func in nc.m.functions:
        for blk in func.blocks:
            new_list = []
            last_sig = None
            n_drop = 0
            for inst in blk.instructions:
                t = type(inst).__name__
                if t == "InstLdweights":
                    s = sig(inst)
                    si = getattr(inst, "sync_info", None)
                    clean = si is None or (
                        not list(si.on_wait) and not list(si.on_update)
                    )
                    if s == last_sig and clean:
                        n_drop += 1
                        continue
                    last_sig = s
                elif t == "InstMatmult":
                    pass  # non-self-loading; weights unchanged
                new_list.append(inst)
            if n_drop:
                blk.instructions = new_list


def _split_multi_waits(nc):
    """Walrus codegen accepts at most one sync-wait per instruction; hoist
    extra waits onto preceding same-engine NoOps (semantically identical —
    the engine stalls at the NoOp instead)."""
    import bass_rust

    for func in nc.m.functions:
        for blk in func.blocks:
            insts = blk.instructions
            new_list = []
            n_split = 0
            for inst in insts:
                si = getattr(inst, "sync_info", None)
                waits = list(si.on_wait) if si is not None else []
                if len(waits) > 1:
                    for k, w in enumerate(waits[:-1]):
                        no = bass_rust.InstNoOp(name=f"{inst.name}-ws{k}")
                        no.engine = inst.engine
                        no.sync_info = bass_rust.SyncInfo(
                            on_wait=[w], on_update=[]
                        )
                        new_list.append(no)
                        n_split += 1
                    inst.sync_info = bass_rust.SyncInfo(
                        on_wait=[waits[-1]], on_update=list(si.on_update)
                    )
                new_list.append(inst)
            if n_split:
                blk.instructions = new_list


@functools.lru_cache(maxsize=8)
def _get_nc(reps=1):
    return _build_nc(reps)


def _host_inputs(x, W):
    """Per-core device input arrays: transposed bf16 x, bf16 W (s-chain),
    fp8 block-diag Cholesky factors (z-chain)."""
    bf16 = ml_dtypes.bfloat16
    fp8 = ml_dtypes.float8_e4m3

    # G = W W^T per (c, r); best rank-MZ PSD approximation via eigh:
    # L6 = V_top sqrt(lam_top), so |x L6|^2 <= x G x^T (exact certificate).
    Wb = W.reshape(C * R, I, O)
    G = Wb @ Wb.transpose(0, 2, 1)          # [C*R, 8, 8]
    lam, V = np.linalg.eigh(G.astype(np.float64))   # ascending
    lam = np.maximum(lam[:, -MZ:], 0.0)
    L6 = (V[:, :, -MZ:] * np.sqrt(lam)[:, None, :]).astype(np.float32)
    L6 = L6.reshape(C, R, I, MZ)            # [c, r, j, m]
    Gs = G.reshape(C, R, I, I)              # [c, r, j, i]
    z_rcs = [rc for rc in range(RCHUNK) if ACT_CHUNK[rc]]
    y_rcs = list(Y_CHUNKS)

    P_IDX = np.arange(128)
    in_maps = []
    for k in range(NCORES):
        rs, re = k * RL, (k + 1) * RL
        xk = x[:, rs:re, :].reshape(B, RCHUNK, 128)
        # xt: [p=(16r,8i), (rc, b)]
        xt_k = np.ascontiguousarray(xk.transpose(2, 1, 0).reshape(128, RCHUNK * B))
        # xs: [b, (y-chunk, 16r, 8i)] — only the y-form chunks' slices
        xs_k = np.ascontiguousarray(xk[:, list(Y_CHUNKS)].reshape(B, -1))

        # ws: [p=(16r,8i), (rc, c, o)]
        wk = (
            W[:, rs:re]
            .reshape(C, RCHUNK, 16, I, O)
            .transpose(1, 2, 3, 0, 4)       # [rc, r, i, c, o]
            .reshape(RCHUNK, 128, C * O)
            .transpose(1, 0, 2)             # [p, rc, c*o]
        )
        ws_k = np.ascontiguousarray(wk.reshape(128, RCHUNK * CO))

        # gb: [p=(16r,8j), per-rc (m, c, r')] block-diagonal factors:
        # rank-MZ eigh factor for z-chunks, G/SCALE_Y for y-chunks
        def _blocks(rc_list, nm, F):
            # F: [c, RCHUNK, 16, j, nm] -> [nrc, 128, nm*C*16] block-diag
            nrc = len(rc_list)
            Fx = (
                F[:, rc_list]
                .transpose(1, 2, 3, 4, 0)   # [rc, r, j, m, c]
                .reshape(nrc, 128, nm, C)
            )
            out = np.zeros((nrc, 128, nm, C, 16), dtype=np.float32)
            out[:, P_IDX, :, :, P_IDX // 8] = Fx.transpose(1, 0, 2, 3)
            return out.reshape(nrc, 128, nm * C * 16)

        zb = _blocks(z_rcs, MZ, L6[:, rs:re].reshape(C, RCHUNK, 16, I, MZ))
        yb = _blocks(
            y_rcs, M,
            Gs[:, rs:re].reshape(C, RCHUNK, 16, I, I) / SCALE_Y)
        gb_f = np.zeros((128, GB_COLS), dtype=np.float32)
        for i, rc in enumerate(z_rcs):
            gb_f[:, GOFF[rc]:GOFF[rc + 1]] = zb[i]
        for i, rc in enumerate(y_rcs):
            gb_f[:, GOFF[rc]:GOFF[rc + 1]] = yb[i]
        gb_k = gb_f.astype(fp8)

        in_maps.append({
            "xt": xt_k.astype(bf16),
            "xs": xs_k.astype(bf16),
            "ws": ws_k.astype(bf16),
            "gb": gb_k,
        })
    return in_maps


def _squash64(s):
    sq = (s * s).sum(-1, keepdims=True)
    return (sq / (1.0 + sq)) * s / np.sqrt(sq)


def kernel(x, route_weights, capsule_bias):
    global LAST_RESULTS
    from concourse.bass_utils import run_bass_kernel_spmd

    x = np.ascontiguousarray(np.asarray(x, dtype=np.float32))
    W = np.ascontiguousarray(np.asarray(route_weights, dtype=np.float32))
    bias = np.asarray(capsule_bias, dtype=np.float64).reshape(C, O)

    in_maps = _host_inputs(x, W)
    try:
        res = run_bass_kernel_spmd(
            _get_nc(), in_maps, core_ids=list(range(NCORES)))
    except ModuleNotFoundError:
        # BASS_TRACE requested but the axon NTFF hook isn't shipped in this
        # container — rerun with tracing disabled.
        import os
        os.environ["BASS_NEVER_TRACE"] = "1"
        res = run_bass_kernel_spmd(
            _get_nc(), in_maps, core_ids=list(range(NCORES)))
    LAST_RESULTS = res
    outs = res.results

    s_sum = np.zeros((B, C, O), dtype=np.float64)
    q2 = np.empty((B, C, R), dtype=np.float32)
    for k in range(NCORES):
        s_sum += np.asarray(outs[k]["s_out"], dtype=np.float64).reshape(B, C, O)
        # v_out: per-rc [b, (m', c, r)] fp8 -> sum over m' -> [b, c, rc*16]
        vk = np.asarray(outs[k]["v_out"]).astype(np.float32)
        for rc in range(RCHUNK):
            nm2 = VW_RC[rc] // (C * 16)
            q2rc = vk[:, VOFF[rc]:VOFF[rc + 1]].reshape(B, nm2, C, 16).sum(1)
            if not ACT_CHUNK[rc]:
                q2rc *= SCALE_Y
            q2[:, :, k * RL + rc * 16:k * RL + (rc + 1) * 16] = q2rc

    # ---- host sparse routing correction (exact, f64) ----
    bs, cs, rs_ = np.nonzero(q2 < THETA2)
    pS = np.einsum(
        "si,sio->so",
        x[bs, rs_].astype(np.float64),
        W[cs, rs_].astype(np.float64),
    )
    qS = (pS * pS).sum(-1)

    Lg = np.zeros(len(bs), dtype=np.float64)
    out = None
    for it in range(ROUTE_ITERATIONS):
        u = np.expm1(Lg)
        usum = np.zeros((B, C))
        np.add.at(usum, (bs, cs), u)
        corr = np.zeros((B, C, O))
        np.add.at(corr, (bs, cs), u[:, None] * pS)
        s = (s_sum + corr) / (R + usum)[..., None]
        out = _squash64(s) + bias[None]
        if it < ROUTE_ITERATIONS - 1:
            outS = out[bs, cs]
            d2 = qS - 2.0 * (pS * outS).sum(-1) + (outS * outS).sum(-1)
            Lg = Lg + np.exp(-d2 / (SIGMA * SIGMA))

    return out.astype(np.float32)
